# revision 11
# baseline (speedup 1.0000x reference)
"""Sliding-window causal GQA attention block (q/k/v proj + RoPE + RMS-norm +
banded softmax attention + output proj) for 8 Trainium2 NeuronCores.

Sharding: batch (2) x kv-head-group (4) -> 8 cores, Megatron-style:
wq/wk/wv column-sharded, wo row-sharded; host sums the 4 row-parallel wo
partials per batch.
"""

import os

import numpy as np

import concourse.bass as bass
import concourse.mybir as mybir
import concourse.tile as tile
from concourse import bacc
from concourse.bass_utils import run_bass_kernel_spmd
from concourse.masks import make_identity

F32 = mybir.dt.float32
F32R = mybir.dt.float32r
BF16 = mybir.dt.bfloat16
AF = mybir.ActivationFunctionType
OP = mybir.AluOpType

B, T, C = 2, 2048, 2048
H, KV, D = 16, 4, 128
G = H // KV            # q heads per kv head (= per core)
WIN = 512              # sliding window
RMS_EPS = 1.1920928955078125e-07
SCALE = 1.0 / np.sqrt(D)
NT = T // 128          # 16 t-tiles
NCC = C // 128         # 16 contraction chunks
NTCH = T // 512        # 4 t-chunks
NEG = -1.0e30

_NC_CACHE = {}
KPHASE = os.environ.get("KPHASE", "full")  # debug: A | full


def _build_nc():
    nc = bacc.Bacc("TRN2", target_bir_lowering=False, debug=False)

    xT = nc.dram_tensor("xT", [C, T], F32R, kind="ExternalInput")
    wqd = nc.dram_tensor("wq", [C, G * D], F32R, kind="ExternalInput")
    wkvd = nc.dram_tensor("wkv", [C, 2 * D], F32R, kind="ExternalInput")
    wod = nc.dram_tensor("wo", [G * D, C], F32R, kind="ExternalInput")
    csd = nc.dram_tensor("cs", [T, D], F32, kind="ExternalInput")
    mlod = nc.dram_tensor("mlo", [128, 128], F32, kind="ExternalInput")
    mhid = nc.dram_tensor("mhi", [128, 128], F32, kind="ExternalInput")
    yd = nc.dram_tensor("y", [T, C], F32, kind="ExternalOutput")

    xTr = xT.rearrange("(cc p) t -> p cc t", p=128)      # [128, 16, T]
    wqr = wqd.rearrange("(cc p) n -> p cc n", p=128)     # [128, 16, 512]
    wkvr = wkvd.rearrange("(cc p) n -> p cc n", p=128)   # [128, 16, 256]
    wor = wod.rearrange("(h p) n -> p h n", p=128)       # [128, 4, 2048]
    csr = csd.rearrange("(n p) d2 -> p n d2", p=128)     # [128, 16, 128]

    with tile.TileContext(nc) as tc:
        with tc.tile_pool(name="outer", bufs=1) as outer:
            qT = outer.tile([128, G, T], F32R, tag="qT")      # [d, h, t]
            kT = outer.tile([128, T], F32R, tag="kT")         # [d, t]
            vsb = outer.tile([128, NT, D], F32R, tag="vsb")   # [s%128, s//128, d]
            cs_sb = outer.tile([128, NT, D], F32, tag="cs")
            mlof = outer.tile([128, 128], F32, tag="mlof")
            mhif = outer.tile([128, 128], F32, tag="mhif")
            mlob = outer.tile([128, 128], BF16, tag="mlob")
            mhib = outer.tile([128, 128], BF16, tag="mhib")
            identf = outer.tile([128, 128], F32, tag="identf")
            ident = outer.tile([128, 128], F32R, tag="ident")
            identb = outer.tile([128, 128], BF16, tag="identb")
            epsb = outer.tile([128, 1], F32, tag="epsb")
            zerof = outer.tile([128, 128], F32, tag="zerof")
            zeros = outer.tile([128, 128], F32R, tag="zeros")

            nc.sync.dma_start(out=cs_sb, in_=csr)
            nc.sync.dma_start(out=mlof, in_=mlod[:, :])
            nc.sync.dma_start(out=mhif, in_=mhid[:, :])
            make_identity(nc, identf)
            nc.vector.tensor_copy(ident, identf)
            nc.vector.tensor_copy(identb, identf)
            nc.vector.tensor_copy(mlob, mlof)
            nc.vector.tensor_copy(mhib, mhif)
            nc.vector.memset(epsb, RMS_EPS)
            nc.vector.memset(zerof, 0.0)
            nc.vector.tensor_copy(zeros, zerof)

            _phase_a(nc, tc, xTr, wqr, wkvr, cs_sb, epsb, ident, qT, kT, vsb)
            if KPHASE == "A":
                with tc.tile_pool(name="dbg", bufs=1) as dbg:
                    d1 = dbg.tile([128, 2048], F32, tag="d1")
                    nc.vector.tensor_copy(d1, qT[:, 0, :].bitcast(F32))
                    nc.sync.dma_start(out=yd[0:128, :], in_=d1)
            else:
                _phase_b(nc, tc, wor, qT, kT, vsb, mlob, mhib, identb, ident,
                         zeros, yd)

    nc.compile()
    return nc


def _phase_a(nc, tc, xTr, wqr, wkvr, cs_sb, epsb, ident, qT, kT, vsb):
    NH = G + 1  # 4 q heads + 1 k head, fused in one psum tile
    with tc.tile_pool(name="wts", bufs=1) as wts, \
         tc.tile_pool(name="xtp", bufs=2) as xtp, \
         tc.tile_pool(name="rtmp", bufs=3) as rtmp, \
         tc.tile_pool(name="qnp", bufs=3) as qnp, \
         tc.tile_pool(name="pqkvp", bufs=2, space="PSUM") as pqkvp, \
         tc.tile_pool(name="ptpA", bufs=2, space="PSUM") as ptpA:

        wq_sb = wts.tile([128, NCC, G * 128], F32R, tag="wq")
        wkv_sb = wts.tile([128, NCC, 2 * 128], F32R, tag="wkv")
        for cc in range(NCC):
            nc.sync.dma_start(out=wq_sb[:, cc, :], in_=wqr[:, cc, :])
            nc.sync.dma_start(out=wkv_sb[:, cc, :], in_=wkvr[:, cc, :])

        for tch in range(NTCH):
            t0c = tch * 512
            xt = xtp.tile([128, NCC, 512], F32R, tag="xt", name="xt")
            for cc in range(NCC):
                nc.sync.dma_start(out=xt[:, cc, :], in_=xTr[:, cc, t0c:t0c + 512])
            for ti in range(4):
                tt = tch * 4 + ti
                o = ti * 128
                # fused q(512) | k(128) | v(128) projection psum [128, 768]
                pqkv = pqkvp.tile([128, 768], F32, tag="pqkv", name="pqkv")
                for cc in range(NCC):
                    nc.tensor.matmul(pqkv[:, 0:512], xt[:, cc, o:o + 128],
                                     wq_sb[:, cc, :],
                                     start=(cc == 0), stop=(cc == NCC - 1))
                for cc in range(NCC):
                    nc.tensor.matmul(pqkv[:, 512:768], xt[:, cc, o:o + 128],
                                     wkv_sb[:, cc, :],
                                     start=(cc == 0), stop=(cc == NCC - 1))
                cs_t = cs_sb[:, tt, :]

                # rope + rms over 5 heads at once (4 q + 1 k)
                pv5 = pqkv[:, 0:NH * 128].rearrange("p (h d) -> p h d", h=NH)
                cos = bass.AP(tensor=cs_t.tensor, offset=cs_t[:, 0:64].offset,
                              ap=[list(cs_t.ap[0]), [0, NH], [1, 64]])
                sin = bass.AP(tensor=cs_t.tensor, offset=cs_t[:, 64:128].offset,
                              ap=[list(cs_t.ap[0]), [0, NH], [1, 64]])
                x1 = pv5[:, :, 0:64]
                x2 = pv5[:, :, 64:128]
                t1 = rtmp.tile([128, NH, 64], F32, tag="t1", name="t1")
                t2 = rtmp.tile([128, NH, 64], F32, tag="t2", name="t2")
                rot = rtmp.tile([128, NH, 128], F32, tag="rot", name="rot")
                nc.vector.tensor_tensor(t1, x1, cos, OP.mult)
                nc.vector.tensor_tensor(t2, x2, sin, OP.mult)
                nc.vector.tensor_tensor(rot[:, :, 0:64], t1, t2, OP.add)
                nc.vector.tensor_tensor(t1, x2, cos, OP.mult)
                nc.vector.tensor_tensor(t2, x1, sin, OP.mult)
                nc.vector.tensor_tensor(rot[:, :, 64:128], t1, t2, OP.subtract)
                sq = rtmp.tile([128, NH, 128], F32, tag="sq", name="sq")
                nc.vector.tensor_tensor(sq, rot, rot, OP.mult)
                ssq = rtmp.tile([128, NH], F32, tag="ssq", name="ssq")
                nc.vector.reduce_sum(ssq, sq, axis=mybir.AxisListType.X)
                nc.scalar.activation(ssq, ssq, AF.Sqrt, bias=epsb, scale=1.0 / D)
                nc.vector.reciprocal(ssq, ssq)
                qn = qnp.tile([128, NH, 128], F32R, tag="qn", name="qn")
                for h in range(NH):
                    nc.vector.tensor_scalar_mul(qn[:, h, :], rot[:, h, :],
                                                ssq[:, h:h + 1])

                # transpose all 5 heads into one [128, 640] psum, batched copy
                pt = ptpA.tile([128, 640], F32R, tag="ptA", name="ptA")
                for h in range(NH):
                    nc.tensor.transpose(pt[:, h * 128:(h + 1) * 128],
                                        qn[:, h, :], ident)
                nc.vector.tensor_copy(
                    qT[:, :, tt * 128:(tt + 1) * 128],
                    pt[:, 0:512].rearrange("p (h c) -> p h c", c=128))
                nc.vector.tensor_copy(kT[:, tt * 128:(tt + 1) * 128],
                                      pt[:, 512:640])
                nc.vector.tensor_copy(vsb[:, tt, :], pqkv[:, 640:768])


def _phase_b(nc, tc, wor, qT, kT, vsb, mlob, mhib, identb, ident, zeros, yd):
    with tc.tile_pool(name="outerB", bufs=1) as outerB:
        att = outerB.tile([128, G, T], F32R, tag="att")   # [d, h, t]
        wo_sb = outerB.tile([128, G, C], F32R, tag="wo")
        for h in range(G):
            nc.sync.dma_start(out=wo_sb[:, h, :], in_=wor[:, h, :])

        with tc.tile_pool(name="pwp", bufs=2) as pwp, \
             tc.tile_pool(name="ep", bufs=3) as ep, \
             tc.tile_pool(name="zp", bufs=4) as zp, \
             tc.tile_pool(name="workp", bufs=3, space="PSUM") as workp, \
             tc.tile_pool(name="paccp", bufs=2, space="PSUM") as paccp:

            for tch in range(NTCH):
                st_lo = max(0, 4 * tch - 4)
                n_st = 4 * tch + 4 - st_lo
                # two pw tiles per chunk; heads (0,2) share pwA, (1,3) pwB,
                # so out-of-band zero cells are filled once per tile
                pwA = pwp.tile([128, 8, 512], F32R, tag="pw", name="pwA")
                pwB = pwp.tile([128, 8, 512], F32R, tag="pw", name="pwB")
                for pw in (pwA, pwB):
                    for sj in range(n_st):
                        for ti in range(4):
                            tt_abs = 4 * tch + ti
                            if not (tt_abs - 4 <= st_lo + sj <= tt_abs):
                                nc.gpsimd.tensor_copy(
                                    pw[:, sj, ti * 128:(ti + 1) * 128], zeros)
                for h in range(G):
                    pw = pwA if h % 2 == 0 else pwB
                    for ti in range(4):
                        tt = tch * 4 + ti
                        t0 = tt * 128
                        w = min(t0 + 128, 640)
                        s0 = max(0, t0 - 512)
                        # scores psum: [128, w] in a [128, 640] work tile,
                        # pieces (512, w-512) to avoid bank crossing
                        sc = workp.tile([128, 640], F32, tag="wk", name="sc")
                        pieces = [(0, min(w, 512))]
                        if w > 512:
                            pieces.append((512, w - 512))
                        for (poff, wp) in pieces:
                            nc.tensor.matmul(
                                sc[:, poff:poff + wp],
                                qT[:, h, t0:t0 + 128],
                                kT[:, s0 + poff:s0 + poff + wp],
                                start=True, stop=False)
                        # masks via bf16 matmul accumulation (PE, not DVE)
                        if t0 >= 512:
                            nc.tensor.matmul(sc[:, 0:128], identb, mlob,
                                             start=False, stop=False,
                                             skip_group_check=True)
                        nc.tensor.matmul(sc[:, w - 128:w], identb, mhib,
                                         start=False, stop=True,
                                         skip_group_check=True)
                        # single exp + row-sum over the full band width
                        E = ep.tile([128, 640], F32, tag="E", name="E")
                        zs = zp.tile([128, 1], F32, tag="zs", name="zs")
                        nc.scalar.activation(E[:, 0:w], sc[:, 0:w], AF.Exp,
                                             scale=float(SCALE),
                                             accum_out=zs)
                        rz = zp.tile([128, 1], F32, tag="rz", name="rz")
                        nc.vector.reciprocal(rz, zs)
                        Er = ep.tile([128, 640], F32R, tag="Er", name="Er")
                        nc.gpsimd.tensor_scalar_mul(Er[:, 0:w], E[:, 0:w], rz)
                        # transpose blocks into one work psum; batched copy out
                        tp = workp.tile([128, 640], F32R, tag="wk", name="tp")
                        nblk = w // 128
                        for bb in range(nblk):
                            nc.tensor.transpose(tp[:, bb * 128:(bb + 1) * 128],
                                                Er[:, bb * 128:(bb + 1) * 128],
                                                ident)
                        sj0 = s0 // 128 - st_lo
                        nc.vector.tensor_copy(
                            pw[:, sj0:sj0 + nblk, ti * 128:(ti + 1) * 128],
                            tp[:, 0:nblk * 128].rearrange(
                                "p (b c) -> p b c", c=128))
                    pO = paccp.tile([128, 512], F32, tag="pacc", name="pO")
                    for sj in range(n_st):
                        nc.tensor.matmul(pO, vsb[:, st_lo + sj, :], pw[:, sj, :],
                                         start=(sj == 0), stop=(sj == n_st - 1))
                    nc.vector.tensor_copy(att[:, h, tch * 512:(tch + 1) * 512],
                                          pO)

        # phase B2: y = attT @ wo (separate dense PE phase)
        with tc.tile_pool(name="ysp", bufs=2) as ysp, \
             tc.tile_pool(name="pYp", bufs=4, space="PSUM") as pYp:
            for tt in range(NT):
                ys = ysp.tile([128, C], F32, tag="ys", name="ys")
                for cc2 in range(4):
                    pY = pYp.tile([128, 512], F32, tag="pY", name="pY")
                    for h in range(G):
                        nc.tensor.matmul(
                            pY, att[:, h, tt * 128:(tt + 1) * 128],
                            wo_sb[:, h, cc2 * 512:(cc2 + 1) * 512],
                            start=(h == 0), stop=(h == G - 1))
                    nc.scalar.copy(ys[:, cc2 * 512:(cc2 + 1) * 512], pY)
                nc.sync.dma_start(out=yd[tt * 128:(tt + 1) * 128, :], in_=ys)


def _get_nc():
    if "nc" not in _NC_CACHE:
        _NC_CACHE["nc"] = _build_nc()
    return _NC_CACHE["nc"]


def _host_inputs(x, cos, sin, wq, wk, wv, wo):
    x = np.asarray(x, dtype=np.float32)
    cos2 = np.asarray(cos, dtype=np.float32).reshape(T, D // 2)
    sin2 = np.asarray(sin, dtype=np.float32).reshape(T, D // 2)
    cs = np.ascontiguousarray(np.concatenate([cos2, sin2], axis=1))
    wq = np.asarray(wq, dtype=np.float32)
    wk = np.asarray(wk, dtype=np.float32)
    wv = np.asarray(wv, dtype=np.float32)
    wo = np.asarray(wo, dtype=np.float32)

    ii = np.arange(128)[:, None]
    jj = np.arange(128)[None, :]
    mlo = np.where(ii <= jj, 0.0, NEG).astype(np.float32)   # keep i <= j
    mhi = np.where(jj <= ii, 0.0, NEG).astype(np.float32)   # keep j <= i

    in_maps = []
    for c in range(8):
        b, g = c // 4, c % 4
        in_maps.append({
            "xT": np.ascontiguousarray(x[b].T),
            "wq": np.ascontiguousarray(wq[:, g * G * D:(g + 1) * G * D]),
            "wkv": np.ascontiguousarray(
                np.concatenate([wk[:, g * D:(g + 1) * D],
                                wv[:, g * D:(g + 1) * D]], axis=1)),
            "wo": np.ascontiguousarray(wo[g * G * D:(g + 1) * G * D, :]),
            "cs": cs,
            "mlo": mlo,
            "mhi": mhi,
        })
    return in_maps


def kernel(x, cos, sin, wq, wk, wv, wo, window_size=512, _trace=False,
           _return_raw=False):
    assert int(window_size) == WIN
    in_maps = _host_inputs(x, cos, sin, wq, wk, wv, wo)
    nc = _get_nc()
    res = run_bass_kernel_spmd(nc, in_maps, list(range(8)), trace=_trace)
    out = np.zeros((B, T, C), dtype=np.float32)
    for c in range(8):
        out[c // 4] += res.results[c]["y"]
    if _return_raw:
        return out, res
    return out


# revision 12
# speedup vs baseline: 2.1932x; 2.1932x over previous
"""Sliding-window causal GQA attention block (q/k/v proj + RoPE + RMS-norm +
banded softmax attention + output proj) for 8 Trainium2 NeuronCores.

Sharding: batch (2) x kv-head-group (4) -> 8 cores, Megatron-style:
wq/wk/wv column-sharded, wo row-sharded; host sums the 4 row-parallel wo
partials per batch.
"""

import os

import numpy as np

import concourse.bass as bass
import concourse.mybir as mybir
import concourse.tile as tile
from concourse import bacc
from concourse.bass_utils import run_bass_kernel_spmd
from concourse.masks import make_identity

F32 = mybir.dt.float32
F32R = mybir.dt.float32r
BF16 = mybir.dt.bfloat16
AF = mybir.ActivationFunctionType
OP = mybir.AluOpType

B, T, C = 2, 2048, 2048
H, KV, D = 16, 4, 128
G = H // KV            # q heads per kv head (= per core)
WIN = 512              # sliding window
RMS_EPS = 1.1920928955078125e-07
SCALE = 1.0 / np.sqrt(D)
NT = T // 128          # 16 t-tiles
NCC = C // 128         # 16 contraction chunks
NTCH = T // 512        # 4 t-chunks
NEG = -1.0e30

_NC_CACHE = {}
KPHASE = os.environ.get("KPHASE", "full")  # debug: A | full


def _build_nc():
    nc = bacc.Bacc("TRN2", target_bir_lowering=False, debug=False)

    xT = nc.dram_tensor("xT", [C, T], F32R, kind="ExternalInput")
    wqd = nc.dram_tensor("wq", [C, G * D], F32R, kind="ExternalInput")
    wkvd = nc.dram_tensor("wkv", [C, 2 * D], F32R, kind="ExternalInput")
    wod = nc.dram_tensor("wo", [G * D, C], F32R, kind="ExternalInput")
    csd = nc.dram_tensor("cs", [T, D], F32, kind="ExternalInput")
    mlod = nc.dram_tensor("mlo", [128, 128], F32, kind="ExternalInput")
    mhid = nc.dram_tensor("mhi", [128, 128], F32, kind="ExternalInput")
    yd = nc.dram_tensor("y", [T, C], F32, kind="ExternalOutput")

    xTr = xT.rearrange("(cc p) t -> p cc t", p=128)      # [128, 16, T]
    wqr = wqd.rearrange("(cc p) n -> p cc n", p=128)     # [128, 16, 512]
    wkvr = wkvd.rearrange("(cc p) n -> p cc n", p=128)   # [128, 16, 256]
    wor = wod.rearrange("(h p) n -> p h n", p=128)       # [128, 4, 2048]
    csr = csd.rearrange("(n p) d2 -> p n d2", p=128)     # [128, 16, 128]

    with tile.TileContext(nc) as tc:
        with tc.tile_pool(name="outer", bufs=1) as outer:
            qT = outer.tile([128, G, T], F32R, tag="qT")      # [d, h, t]
            kT = outer.tile([128, T], F32R, tag="kT")         # [d, t]
            vsb = outer.tile([128, NT, D], F32R, tag="vsb")   # [s%128, s//128, d]
            cs_sb = outer.tile([128, NT, D], F32, tag="cs")
            mlof = outer.tile([128, 128], F32, tag="mlof")
            mhif = outer.tile([128, 128], F32, tag="mhif")
            mlob = outer.tile([128, 128], BF16, tag="mlob")
            mhib = outer.tile([128, 128], BF16, tag="mhib")
            identf = outer.tile([128, 128], F32, tag="identf")
            ident = outer.tile([128, 128], F32R, tag="ident")
            identb = outer.tile([128, 128], BF16, tag="identb")
            epsb = outer.tile([128, 1], F32, tag="epsb")
            zerof = outer.tile([128, 128], F32, tag="zerof")
            zeros = outer.tile([128, 128], F32R, tag="zeros")

            nc.sync.dma_start(out=cs_sb, in_=csr)
            nc.sync.dma_start(out=mlof, in_=mlod[:, :])
            nc.sync.dma_start(out=mhif, in_=mhid[:, :])
            make_identity(nc, identf)
            nc.vector.tensor_copy(ident, identf)
            nc.vector.tensor_copy(identb, identf)
            nc.vector.tensor_copy(mlob, mlof)
            nc.vector.tensor_copy(mhib, mhif)
            nc.vector.memset(epsb, RMS_EPS)
            nc.vector.memset(zerof, 0.0)
            nc.vector.tensor_copy(zeros, zerof)

            _phase_a(nc, tc, xTr, wqr, wkvr, cs_sb, epsb, ident, qT, kT, vsb)
            if KPHASE == "A":
                with tc.tile_pool(name="dbg", bufs=1) as dbg:
                    d1 = dbg.tile([128, 2048], F32, tag="d1")
                    nc.vector.tensor_copy(d1, qT[:, 0, :].bitcast(F32))
                    nc.sync.dma_start(out=yd[0:128, :], in_=d1)
            else:
                _phase_b(nc, tc, wor, qT, kT, vsb, mlob, mhib, identb, ident,
                         zeros, yd)

    nc.compile()
    return nc


def _phase_a(nc, tc, xTr, wqr, wkvr, cs_sb, epsb, ident, qT, kT, vsb):
    NH = G + 1  # 4 q heads + 1 k head, fused in one psum tile
    with tc.tile_pool(name="wts", bufs=1) as wts, \
         tc.tile_pool(name="xtp", bufs=2) as xtp, \
         tc.tile_pool(name="rtmp", bufs=3) as rtmp, \
         tc.tile_pool(name="qnp", bufs=3) as qnp, \
         tc.tile_pool(name="pqkvp", bufs=2, space="PSUM") as pqkvp, \
         tc.tile_pool(name="ptpA", bufs=2, space="PSUM") as ptpA:

        wq_sb = wts.tile([128, NCC, G * 128], F32R, tag="wq")
        wkv_sb = wts.tile([128, NCC, 2 * 128], F32R, tag="wkv")
        for cc in range(NCC):
            nc.sync.dma_start(out=wq_sb[:, cc, :], in_=wqr[:, cc, :])
            nc.sync.dma_start(out=wkv_sb[:, cc, :], in_=wkvr[:, cc, :])

        for tch in range(NTCH):
            t0c = tch * 512
            xt = xtp.tile([128, NCC, 512], F32R, tag="xt", name="xt")
            for cc in range(NCC):
                nc.sync.dma_start(out=xt[:, cc, :], in_=xTr[:, cc, t0c:t0c + 512])
            for ti in range(4):
                tt = tch * 4 + ti
                o = ti * 128
                # fused q(512) | k(128) | v(128) projection psum [128, 768]
                pqkv = pqkvp.tile([128, 768], F32, tag="pqkv", name="pqkv")
                for cc in range(NCC):
                    nc.tensor.matmul(pqkv[:, 0:512], xt[:, cc, o:o + 128],
                                     wq_sb[:, cc, :],
                                     start=(cc == 0), stop=(cc == NCC - 1))
                for cc in range(NCC):
                    nc.tensor.matmul(pqkv[:, 512:768], xt[:, cc, o:o + 128],
                                     wkv_sb[:, cc, :],
                                     start=(cc == 0), stop=(cc == NCC - 1))
                cs_t = cs_sb[:, tt, :]

                # rope + rms over 5 heads at once (4 q + 1 k)
                pv5 = pqkv[:, 0:NH * 128].rearrange("p (h d) -> p h d", h=NH)
                cos = bass.AP(tensor=cs_t.tensor, offset=cs_t[:, 0:64].offset,
                              ap=[list(cs_t.ap[0]), [0, NH], [1, 64]])
                sin = bass.AP(tensor=cs_t.tensor, offset=cs_t[:, 64:128].offset,
                              ap=[list(cs_t.ap[0]), [0, NH], [1, 64]])
                x1 = pv5[:, :, 0:64]
                x2 = pv5[:, :, 64:128]
                t1 = rtmp.tile([128, NH, 64], F32, tag="t1", name="t1")
                t2 = rtmp.tile([128, NH, 64], F32, tag="t2", name="t2")
                rot = rtmp.tile([128, NH, 128], F32, tag="rot", name="rot")
                nc.vector.tensor_tensor(t1, x1, cos, OP.mult)
                nc.vector.tensor_tensor(t2, x2, sin, OP.mult)
                nc.vector.tensor_tensor(rot[:, :, 0:64], t1, t2, OP.add)
                nc.vector.tensor_tensor(t1, x2, cos, OP.mult)
                nc.vector.tensor_tensor(t2, x1, sin, OP.mult)
                nc.vector.tensor_tensor(rot[:, :, 64:128], t1, t2, OP.subtract)
                sq = rtmp.tile([128, NH, 128], F32, tag="sq", name="sq")
                nc.vector.tensor_tensor(sq, rot, rot, OP.mult)
                ssq = rtmp.tile([128, NH], F32, tag="ssq", name="ssq")
                nc.vector.reduce_sum(ssq, sq, axis=mybir.AxisListType.X)
                nc.scalar.activation(ssq, ssq, AF.Sqrt, bias=epsb, scale=1.0 / D)
                nc.vector.reciprocal(ssq, ssq)
                qn = qnp.tile([128, NH, 128], F32R, tag="qn", name="qn")
                for h in range(NH):
                    nc.vector.tensor_scalar_mul(qn[:, h, :], rot[:, h, :],
                                                ssq[:, h:h + 1])

                # transpose all 5 heads into one [128, 640] psum, batched copy
                pt = ptpA.tile([128, 640], F32R, tag="ptA", name="ptA")
                for h in range(NH):
                    nc.tensor.transpose(pt[:, h * 128:(h + 1) * 128],
                                        qn[:, h, :], ident)
                nc.vector.tensor_copy(
                    qT[:, :, tt * 128:(tt + 1) * 128],
                    pt[:, 0:512].rearrange("p (h c) -> p h c", c=128))
                nc.vector.tensor_copy(kT[:, tt * 128:(tt + 1) * 128],
                                      pt[:, 512:640])
                nc.vector.tensor_copy(vsb[:, tt, :], pqkv[:, 640:768])


def _phase_b(nc, tc, wor, qT, kT, vsb, mlob, mhib, identb, ident, zeros, yd):
    with tc.tile_pool(name="outerB", bufs=1) as outerB:
        att = outerB.tile([128, G, T], F32R, tag="att")   # [d, h, t]
        wo_sb = outerB.tile([128, G, C], F32R, tag="wo")
        for h in range(G):
            nc.sync.dma_start(out=wo_sb[:, h, :], in_=wor[:, h, :])

        with tc.tile_pool(name="pwp", bufs=2) as pwp, \
             tc.tile_pool(name="ep", bufs=3) as ep, \
             tc.tile_pool(name="zp", bufs=4) as zp, \
             tc.tile_pool(name="workp", bufs=3, space="PSUM") as workp, \
             tc.tile_pool(name="paccp", bufs=2, space="PSUM") as paccp:

            for tch in range(NTCH):
                st_lo = max(0, 4 * tch - 4)
                n_st = 4 * tch + 4 - st_lo
                # two pw tiles per chunk; heads (0,2) share pwA, (1,3) pwB,
                # so out-of-band zero cells are filled once per tile
                pwA = pwp.tile([128, 8, 512], F32R, tag="pw", name="pwA")
                pwB = pwp.tile([128, 8, 512], F32R, tag="pw", name="pwB")
                for pw in (pwA, pwB):
                    for sj in range(n_st):
                        for ti in range(4):
                            tt_abs = 4 * tch + ti
                            if not (tt_abs - 4 <= st_lo + sj <= tt_abs):
                                nc.gpsimd.tensor_copy(
                                    pw[:, sj, ti * 128:(ti + 1) * 128], zeros)
                for h in range(G):
                    pw = pwA if h % 2 == 0 else pwB
                    for ti in range(4):
                        tt = tch * 4 + ti
                        t0 = tt * 128
                        w = min(t0 + 128, 640)
                        s0 = max(0, t0 - 512)
                        # scores psum: [128, w] in a [128, 640] work tile,
                        # pieces (512, w-512) to avoid bank crossing
                        sc = workp.tile([128, 640], F32, tag="wk", name="sc")
                        pieces = [(0, min(w, 512))]
                        if w > 512:
                            pieces.append((512, w - 512))
                        for (poff, wp) in pieces:
                            nc.tensor.matmul(
                                sc[:, poff:poff + wp],
                                qT[:, h, t0:t0 + 128],
                                kT[:, s0 + poff:s0 + poff + wp],
                                start=True, stop=False)
                        # masks via bf16 matmul accumulation (PE, not DVE)
                        if t0 >= 512:
                            nc.tensor.matmul(sc[:, 0:128], identb, mlob,
                                             start=False, stop=False,
                                             skip_group_check=True)
                        nc.tensor.matmul(sc[:, w - 128:w], identb, mhib,
                                         start=False, stop=True,
                                         skip_group_check=True)
                        # single exp + row-sum over the full band width
                        E = ep.tile([128, 640], F32, tag="E", name="E")
                        zs = zp.tile([128, 1], F32, tag="zs", name="zs")
                        nc.scalar.activation(E[:, 0:w], sc[:, 0:w], AF.Exp,
                                             scale=float(SCALE),
                                             accum_out=zs)
                        rz = zp.tile([128, 1], F32, tag="rz", name="rz")
                        nc.vector.reciprocal(rz, zs)
                        Er = ep.tile([128, 640], F32R, tag="Er", name="Er")
                        nc.vector.tensor_scalar_mul(Er[:, 0:w], E[:, 0:w], rz)
                        # transpose blocks into one work psum; batched copy out
                        tp = workp.tile([128, 640], F32R, tag="wk", name="tp")
                        nblk = w // 128
                        for bb in range(nblk):
                            nc.tensor.transpose(tp[:, bb * 128:(bb + 1) * 128],
                                                Er[:, bb * 128:(bb + 1) * 128],
                                                ident)
                        sj0 = s0 // 128 - st_lo
                        nc.vector.tensor_copy(
                            pw[:, sj0:sj0 + nblk, ti * 128:(ti + 1) * 128],
                            tp[:, 0:nblk * 128].rearrange(
                                "p (b c) -> p b c", c=128))
                    pO = paccp.tile([128, 512], F32, tag="pacc", name="pO")
                    for sj in range(n_st):
                        nc.tensor.matmul(pO, vsb[:, st_lo + sj, :], pw[:, sj, :],
                                         start=(sj == 0), stop=(sj == n_st - 1))
                    nc.vector.tensor_copy(att[:, h, tch * 512:(tch + 1) * 512],
                                          pO)

        # phase B2: y = attT @ wo (separate dense PE phase)
        with tc.tile_pool(name="ysp", bufs=2) as ysp, \
             tc.tile_pool(name="pYp", bufs=4, space="PSUM") as pYp:
            for tt in range(NT):
                ys = ysp.tile([128, C], F32, tag="ys", name="ys")
                for cc2 in range(4):
                    pY = pYp.tile([128, 512], F32, tag="pY", name="pY")
                    for h in range(G):
                        nc.tensor.matmul(
                            pY, att[:, h, tt * 128:(tt + 1) * 128],
                            wo_sb[:, h, cc2 * 512:(cc2 + 1) * 512],
                            start=(h == 0), stop=(h == G - 1))
                    nc.scalar.copy(ys[:, cc2 * 512:(cc2 + 1) * 512], pY)
                nc.sync.dma_start(out=yd[tt * 128:(tt + 1) * 128, :], in_=ys)


def _get_nc():
    if "nc" not in _NC_CACHE:
        _NC_CACHE["nc"] = _build_nc()
    return _NC_CACHE["nc"]


def _host_inputs(x, cos, sin, wq, wk, wv, wo):
    x = np.asarray(x, dtype=np.float32)
    cos2 = np.asarray(cos, dtype=np.float32).reshape(T, D // 2)
    sin2 = np.asarray(sin, dtype=np.float32).reshape(T, D // 2)
    cs = np.ascontiguousarray(np.concatenate([cos2, sin2], axis=1))
    wq = np.asarray(wq, dtype=np.float32)
    wk = np.asarray(wk, dtype=np.float32)
    wv = np.asarray(wv, dtype=np.float32)
    wo = np.asarray(wo, dtype=np.float32)

    ii = np.arange(128)[:, None]
    jj = np.arange(128)[None, :]
    mlo = np.where(ii <= jj, 0.0, NEG).astype(np.float32)   # keep i <= j
    mhi = np.where(jj <= ii, 0.0, NEG).astype(np.float32)   # keep j <= i

    in_maps = []
    for c in range(8):
        b, g = c // 4, c % 4
        in_maps.append({
            "xT": np.ascontiguousarray(x[b].T),
            "wq": np.ascontiguousarray(wq[:, g * G * D:(g + 1) * G * D]),
            "wkv": np.ascontiguousarray(
                np.concatenate([wk[:, g * D:(g + 1) * D],
                                wv[:, g * D:(g + 1) * D]], axis=1)),
            "wo": np.ascontiguousarray(wo[g * G * D:(g + 1) * G * D, :]),
            "cs": cs,
            "mlo": mlo,
            "mhi": mhi,
        })
    return in_maps


def kernel(x, cos, sin, wq, wk, wv, wo, window_size=512, _trace=False,
           _return_raw=False):
    assert int(window_size) == WIN
    in_maps = _host_inputs(x, cos, sin, wq, wk, wv, wo)
    nc = _get_nc()
    res = run_bass_kernel_spmd(nc, in_maps, list(range(8)), trace=_trace)
    out = np.zeros((B, T, C), dtype=np.float32)
    for c in range(8):
        out[c // 4] += res.results[c]["y"]
    if _return_raw:
        return out, res
    return out


# revision 13
# speedup vs baseline: 3.1415x; 1.4324x over previous
"""Sliding-window causal GQA attention block (q/k/v proj + RoPE + RMS-norm +
banded softmax attention + output proj) for 8 Trainium2 NeuronCores.

Sharding: batch (2) x kv-head-group (4) -> 8 cores, Megatron-style:
wq/wk/wv column-sharded, wo row-sharded; host sums the 4 row-parallel wo
partials per batch.
"""

import os

import numpy as np

import concourse.bass as bass
import concourse.mybir as mybir
import concourse.tile as tile
from concourse import bacc
from concourse.bass_utils import run_bass_kernel_spmd
from concourse.masks import make_identity

F32 = mybir.dt.float32
F32R = mybir.dt.float32r
BF16 = mybir.dt.bfloat16
AF = mybir.ActivationFunctionType
OP = mybir.AluOpType

B, T, C = 2, 2048, 2048
H, KV, D = 16, 4, 128
G = H // KV            # q heads per kv head (= per core)
WIN = 512              # sliding window
RMS_EPS = 1.1920928955078125e-07
SCALE = 1.0 / np.sqrt(D)
NT = T // 128          # 16 t-tiles
NCC = C // 128         # 16 contraction chunks
NTCH = T // 512        # 4 t-chunks
NEG = -1.0e30

_NC_CACHE = {}
KPHASE = os.environ.get("KPHASE", "full")  # debug: A | full


def _build_nc():
    nc = bacc.Bacc("TRN2", target_bir_lowering=False, debug=False)

    xT = nc.dram_tensor("xT", [C, T], F32R, kind="ExternalInput")
    wqd = nc.dram_tensor("wq", [C, G * D], F32R, kind="ExternalInput")
    wkvd = nc.dram_tensor("wkv", [C, 2 * D], F32R, kind="ExternalInput")
    wod = nc.dram_tensor("wo", [G * D, C], F32R, kind="ExternalInput")
    csd = nc.dram_tensor("cs", [T, D], F32, kind="ExternalInput")
    mlod = nc.dram_tensor("mlo", [128, 128], F32, kind="ExternalInput")
    mhid = nc.dram_tensor("mhi", [128, 128], F32, kind="ExternalInput")
    yd = nc.dram_tensor("y", [T, C], F32, kind="ExternalOutput")

    xTr = xT.rearrange("(cc p) t -> p cc t", p=128)      # [128, 16, T]
    wqr = wqd.rearrange("(cc p) n -> p cc n", p=128)     # [128, 16, 512]
    wkvr = wkvd.rearrange("(cc p) n -> p cc n", p=128)   # [128, 16, 256]
    wor = wod.rearrange("(h p) n -> p h n", p=128)       # [128, 4, 2048]
    csr = csd.rearrange("(n p) d2 -> p n d2", p=128)     # [128, 16, 128]

    with tile.TileContext(nc) as tc:
        with tc.tile_pool(name="outer", bufs=1) as outer:
            qT = outer.tile([128, G, T], F32R, tag="qT")      # [d, h, t]
            kT = outer.tile([128, T], F32R, tag="kT")         # [d, t]
            vsb = outer.tile([128, NT, D], BF16, tag="vsb")   # [s%128, s//128, d]
            cs_sb = outer.tile([128, NT, D], F32, tag="cs")
            mlof = outer.tile([128, 128], F32, tag="mlof")
            mhif = outer.tile([128, 128], F32, tag="mhif")
            mlob = outer.tile([128, 128], BF16, tag="mlob")
            mhib = outer.tile([128, 128], BF16, tag="mhib")
            identf = outer.tile([128, 128], F32, tag="identf")
            ident = outer.tile([128, 128], F32R, tag="ident")
            identb = outer.tile([128, 128], BF16, tag="identb")
            epsb = outer.tile([128, 1], F32, tag="epsb")
            zerof = outer.tile([128, 128], F32, tag="zerof")
            zeros = outer.tile([128, 128], BF16, tag="zeros")

            nc.sync.dma_start(out=cs_sb, in_=csr)
            nc.sync.dma_start(out=mlof, in_=mlod[:, :])
            nc.sync.dma_start(out=mhif, in_=mhid[:, :])
            make_identity(nc, identf)
            nc.vector.tensor_copy(ident, identf)
            nc.vector.tensor_copy(identb, identf)
            nc.vector.tensor_copy(mlob, mlof)
            nc.vector.tensor_copy(mhib, mhif)
            nc.vector.memset(epsb, RMS_EPS)
            nc.vector.memset(zerof, 0.0)
            nc.vector.tensor_copy(zeros, zerof)

            _phase_a(nc, tc, xTr, wqr, wkvr, cs_sb, epsb, ident, qT, kT, vsb)
            if KPHASE == "A":
                with tc.tile_pool(name="dbg", bufs=1) as dbg:
                    d1 = dbg.tile([128, 2048], F32, tag="d1")
                    nc.vector.tensor_copy(d1, qT[:, 0, :].bitcast(F32))
                    nc.sync.dma_start(out=yd[0:128, :], in_=d1)
            else:
                _phase_b(nc, tc, wor, qT, kT, vsb, mlob, mhib, identb, ident,
                         zeros, yd)

    nc.compile()
    return nc


def _phase_a(nc, tc, xTr, wqr, wkvr, cs_sb, epsb, ident, qT, kT, vsb):
    NH = G + 1  # 4 q heads + 1 k head, fused in one psum tile
    with tc.tile_pool(name="wts", bufs=1) as wts, \
         tc.tile_pool(name="xtp", bufs=2) as xtp, \
         tc.tile_pool(name="rtmp", bufs=3) as rtmp, \
         tc.tile_pool(name="qnp", bufs=3) as qnp, \
         tc.tile_pool(name="pqkvp", bufs=2, space="PSUM") as pqkvp, \
         tc.tile_pool(name="ptpA", bufs=2, space="PSUM") as ptpA:

        wq_sb = wts.tile([128, NCC, G * 128], F32R, tag="wq")
        wkv_sb = wts.tile([128, NCC, 2 * 128], F32R, tag="wkv")
        xt0 = xtp.tile([128, NCC, 512], F32R, tag="xt", name="xt0")
        for cc in range(NCC):
            nc.sync.dma_start(out=wq_sb[:, cc, :], in_=wqr[:, cc, :])
            nc.sync.dma_start(out=xt0[:, cc, :], in_=xTr[:, cc, 0:512])
            nc.sync.dma_start(out=wkv_sb[:, cc, :], in_=wkvr[:, cc, :])

        for tch in range(NTCH):
            t0c = tch * 512
            if tch == 0:
                xt = xt0
            else:
                xt = xtp.tile([128, NCC, 512], F32R, tag="xt", name="xt")
                for cc in range(NCC):
                    nc.sync.dma_start(out=xt[:, cc, :],
                                      in_=xTr[:, cc, t0c:t0c + 512])
            for ti in range(4):
                tt = tch * 4 + ti
                o = ti * 128
                # fused q(512) | k(128) | v(128) projection psum [128, 768]
                pqkv = pqkvp.tile([128, 768], F32, tag="pqkv", name="pqkv")
                for cc in range(NCC):
                    nc.tensor.matmul(pqkv[:, 0:512], xt[:, cc, o:o + 128],
                                     wq_sb[:, cc, :],
                                     start=(cc == 0), stop=(cc == NCC - 1))
                for cc in range(NCC):
                    nc.tensor.matmul(pqkv[:, 512:768], xt[:, cc, o:o + 128],
                                     wkv_sb[:, cc, :],
                                     start=(cc == 0), stop=(cc == NCC - 1))
                cs_t = cs_sb[:, tt, :]

                # rope + rms over 5 heads at once (4 q + 1 k)
                pv5 = pqkv[:, 0:NH * 128].rearrange("p (h d) -> p h d", h=NH)
                cos = bass.AP(tensor=cs_t.tensor, offset=cs_t[:, 0:64].offset,
                              ap=[list(cs_t.ap[0]), [0, NH], [1, 64]])
                sin = bass.AP(tensor=cs_t.tensor, offset=cs_t[:, 64:128].offset,
                              ap=[list(cs_t.ap[0]), [0, NH], [1, 64]])
                x1 = pv5[:, :, 0:64]
                x2 = pv5[:, :, 64:128]
                t1 = rtmp.tile([128, NH, 64], F32, tag="t1", name="t1")
                t2 = rtmp.tile([128, NH, 64], F32, tag="t2", name="t2")
                rot = rtmp.tile([128, NH, 128], F32, tag="rot", name="rot")
                nc.vector.tensor_tensor(t1, x1, cos, OP.mult)
                nc.vector.tensor_tensor(t2, x2, sin, OP.mult)
                nc.vector.tensor_tensor(rot[:, :, 0:64], t1, t2, OP.add)
                nc.vector.tensor_tensor(t1, x2, cos, OP.mult)
                nc.vector.tensor_tensor(t2, x1, sin, OP.mult)
                nc.vector.tensor_tensor(rot[:, :, 64:128], t1, t2, OP.subtract)
                sq = rtmp.tile([128, NH, 128], F32, tag="sq", name="sq")
                nc.vector.tensor_tensor(sq, rot, rot, OP.mult)
                ssq = rtmp.tile([128, NH], F32, tag="ssq", name="ssq")
                nc.vector.reduce_sum(ssq, sq, axis=mybir.AxisListType.X)
                nc.scalar.activation(ssq, ssq, AF.Sqrt, bias=epsb, scale=1.0 / D)
                nc.vector.reciprocal(ssq, ssq)
                qn = qnp.tile([128, NH, 128], F32R, tag="qn", name="qn")
                for h in range(NH):
                    nc.vector.tensor_scalar_mul(qn[:, h, :], rot[:, h, :],
                                                ssq[:, h:h + 1])

                # transpose all 5 heads into one [128, 640] psum, batched copy
                pt = ptpA.tile([128, 640], F32R, tag="ptA", name="ptA")
                for h in range(NH):
                    nc.tensor.transpose(pt[:, h * 128:(h + 1) * 128],
                                        qn[:, h, :], ident)
                nc.vector.tensor_copy(
                    qT[:, :, tt * 128:(tt + 1) * 128],
                    pt[:, 0:512].rearrange("p (h c) -> p h c", c=128))
                nc.vector.tensor_copy(kT[:, tt * 128:(tt + 1) * 128],
                                      pt[:, 512:640])
                nc.vector.tensor_copy(vsb[:, tt, :], pqkv[:, 640:768])


def _phase_b(nc, tc, wor, qT, kT, vsb, mlob, mhib, identb, ident, zeros, yd):
    with tc.tile_pool(name="outerB", bufs=1) as outerB:
        att = outerB.tile([128, G, T], F32R, tag="att")   # [d, h, t]
        wo_sb = outerB.tile([128, G, C], F32R, tag="wo")
        for h in range(G):
            nc.sync.dma_start(out=wo_sb[:, h, :], in_=wor[:, h, :])

        with tc.tile_pool(name="pwp", bufs=2) as pwp, \
             tc.tile_pool(name="ep", bufs=3) as ep, \
             tc.tile_pool(name="zp", bufs=4) as zp, \
             tc.tile_pool(name="scp", bufs=2, space="PSUM") as scp, \
             tc.tile_pool(name="tpp", bufs=2, space="PSUM") as tpp, \
             tc.tile_pool(name="paccp", bufs=2, space="PSUM") as paccp:

            for tch in range(NTCH):
                st_lo = max(0, 4 * tch - 4)
                n_st = 4 * tch + 4 - st_lo
                # two pw tiles per chunk; heads (0,2) share pwA, (1,3) pwB,
                # so out-of-band zero cells are filled once per tile
                pwA = pwp.tile([128, 8, 512], BF16, tag="pw", name="pwA")
                pwB = pwp.tile([128, 8, 512], BF16, tag="pw", name="pwB")
                for pw in (pwA, pwB):
                    for sj in range(n_st):
                        for ti in range(4):
                            tt_abs = 4 * tch + ti
                            if not (tt_abs - 4 <= st_lo + sj <= tt_abs):
                                nc.gpsimd.tensor_copy(
                                    pw[:, sj, ti * 128:(ti + 1) * 128], zeros)
                for h in range(G):
                    pw = pwA if h % 2 == 0 else pwB
                    for ti in range(4):
                        tt = tch * 4 + ti
                        t0 = tt * 128
                        w = min(t0 + 128, 640)
                        s0 = max(0, t0 - 512)
                        # scores psum: [128, w] in a [128, 640] work tile,
                        # pieces (512, w-512) to avoid bank crossing
                        sc = scp.tile([128, 640], F32, tag="sc", name="sc")
                        pieces = [(0, min(w, 512))]
                        if w > 512:
                            pieces.append((512, w - 512))
                        for (poff, wp) in pieces:
                            nc.tensor.matmul(
                                sc[:, poff:poff + wp],
                                qT[:, h, t0:t0 + 128],
                                kT[:, s0 + poff:s0 + poff + wp],
                                start=True, stop=False)
                        # masks via bf16 matmul accumulation (PE, not DVE)
                        if t0 >= 512:
                            nc.tensor.matmul(sc[:, 0:128], identb, mlob,
                                             start=False, stop=False,
                                             skip_group_check=True)
                        nc.tensor.matmul(sc[:, w - 128:w], identb, mhib,
                                         start=False, stop=True,
                                         skip_group_check=True)
                        # single exp + row-sum over the full band width
                        E = ep.tile([128, 640], F32, tag="E", name="E")
                        zs = zp.tile([128, 1], F32, tag="zs", name="zs")
                        nc.scalar.activation(E[:, 0:w], sc[:, 0:w], AF.Exp,
                                             scale=float(SCALE),
                                             accum_out=zs)
                        rz = zp.tile([128, 1], F32, tag="rz", name="rz")
                        nc.vector.reciprocal(rz, zs)
                        Er = ep.tile([128, 640], BF16, tag="Er", name="Er")
                        nc.vector.tensor_scalar_mul(Er[:, 0:w], E[:, 0:w], rz)
                        # transpose blocks into one work psum; batched copy out
                        tp = tpp.tile([128, 640], BF16, tag="tp", name="tp")
                        nblk = w // 128
                        for bb in range(nblk):
                            nc.tensor.transpose(tp[:, bb * 128:(bb + 1) * 128],
                                                Er[:, bb * 128:(bb + 1) * 128],
                                                identb)
                        sj0 = s0 // 128 - st_lo
                        nc.vector.tensor_copy(
                            pw[:, sj0:sj0 + nblk, ti * 128:(ti + 1) * 128],
                            tp[:, 0:nblk * 128].rearrange(
                                "p (b c) -> p b c", c=128))
                    pO = paccp.tile([128, 512], F32, tag="pacc", name="pO")
                    for sj in range(n_st):
                        nc.tensor.matmul(pO, vsb[:, st_lo + sj, :], pw[:, sj, :],
                                         start=(sj == 0), stop=(sj == n_st - 1))
                    nc.vector.tensor_copy(att[:, h, tch * 512:(tch + 1) * 512],
                                          pO)

        # phase B2: y = attT @ wo (separate dense PE phase)
        with tc.tile_pool(name="ysp", bufs=2) as ysp, \
             tc.tile_pool(name="pYp", bufs=4, space="PSUM") as pYp:
            for tt in range(NT):
                ys = ysp.tile([128, C], F32, tag="ys", name="ys")
                for cc2 in range(4):
                    pY = pYp.tile([128, 512], F32, tag="pY", name="pY")
                    for h in range(G):
                        nc.tensor.matmul(
                            pY, att[:, h, tt * 128:(tt + 1) * 128],
                            wo_sb[:, h, cc2 * 512:(cc2 + 1) * 512],
                            start=(h == 0), stop=(h == G - 1))
                    nc.scalar.copy(ys[:, cc2 * 512:(cc2 + 1) * 512], pY)
                nc.sync.dma_start(out=yd[tt * 128:(tt + 1) * 128, :], in_=ys)


def _get_nc():
    if "nc" not in _NC_CACHE:
        _NC_CACHE["nc"] = _build_nc()
    return _NC_CACHE["nc"]


def _host_inputs(x, cos, sin, wq, wk, wv, wo):
    x = np.asarray(x, dtype=np.float32)
    cos2 = np.asarray(cos, dtype=np.float32).reshape(T, D // 2)
    sin2 = np.asarray(sin, dtype=np.float32).reshape(T, D // 2)
    cs = np.ascontiguousarray(np.concatenate([cos2, sin2], axis=1))
    wq = np.asarray(wq, dtype=np.float32)
    wk = np.asarray(wk, dtype=np.float32)
    wv = np.asarray(wv, dtype=np.float32)
    wo = np.asarray(wo, dtype=np.float32)

    ii = np.arange(128)[:, None]
    jj = np.arange(128)[None, :]
    mlo = np.where(ii <= jj, 0.0, NEG).astype(np.float32)   # keep i <= j
    mhi = np.where(jj <= ii, 0.0, NEG).astype(np.float32)   # keep j <= i

    in_maps = []
    for c in range(8):
        b, g = c // 4, c % 4
        in_maps.append({
            "xT": np.ascontiguousarray(x[b].T),
            "wq": np.ascontiguousarray(wq[:, g * G * D:(g + 1) * G * D]),
            "wkv": np.ascontiguousarray(
                np.concatenate([wk[:, g * D:(g + 1) * D],
                                wv[:, g * D:(g + 1) * D]], axis=1)),
            "wo": np.ascontiguousarray(wo[g * G * D:(g + 1) * G * D, :]),
            "cs": cs,
            "mlo": mlo,
            "mhi": mhi,
        })
    return in_maps


def kernel(x, cos, sin, wq, wk, wv, wo, window_size=512, _trace=False,
           _return_raw=False):
    assert int(window_size) == WIN
    in_maps = _host_inputs(x, cos, sin, wq, wk, wv, wo)
    nc = _get_nc()
    res = run_bass_kernel_spmd(nc, in_maps, list(range(8)), trace=_trace)
    out = np.zeros((B, T, C), dtype=np.float32)
    for c in range(8):
        out[c // 4] += res.results[c]["y"]
    if _return_raw:
        return out, res
    return out
